# revision 26
# baseline (speedup 1.0000x reference)
"""AdditiveAttention kernel for one TRN2 chip (8 NeuronCores).

Reference computation (per batch b):
    q = queries @ W_q                         # (NQ, H)
    k = keys @ W_k                            # (NK, H)
    scores[i,j] = sum_h v_w[h] * tanh(q[i,h] + k[j,h])
    out = masked_softmax(scores, valid_len) @ values

Sharding: data-parallel over (batch, query-half): core c handles batch c//2,
query rows (c%2)*64 .. +64.  All compute is core-local (no collectives);
the host does layout prep (transposes / masking / padding) and reassembly.

Device dataflow per core (64 queries x 1024 keys x H=256):
  P1  kh[h,j] = W_k.T @ keys.T, qh[h,i] = W_q.T @ queries.T     (PE, bf16)
  P2  sums = kh + qh[:,i]   (VectorE tensor_scalar, bf16 4x mode)
      feat = tanh(sums)     (ScalarE, giant FD to amortize overhead)
      scoresT[j,i] += feat[h,jblk].T @ v_w  (PE, feat as stationary operand)
      -> scores accumulate TRANSPOSED in PSUM, [j,i] layout
  P3  wT = exp(scoresT)  (no max subtraction needed: |score| <= sum|v_w| ~ 13)
  P4  out_aug[i,:] = sum_j wT[j,i] * va[j,:]  where va = [masked values | mask]
      -> column 256 of out_aug is the softmax denominator
  P5  out = out_aug[:, :256] * (1 / out_aug[:, 256])
"""

import numpy as np
import ml_dtypes

import concourse.bass as bass
import concourse.tile as tile
from concourse import bacc, mybir
from concourse.bass_utils import run_bass_kernel_spmd

BF16 = mybir.dt.bfloat16
F32 = mybir.dt.float32
NP_BF16 = ml_dtypes.bfloat16

B, NQ, NK, DQ, DK, H, DV = 4, 128, 1024, 256, 256, 256, 256
NQC = NQ // 2  # queries per core
VA_W = 258  # values (256) + mask column (1) + pad (1)
N_CORES = 8
IB = 8  # queries per tanh block

# packed "ck{dt}" layout (per partition row p = d % 128, bf16):
#   [ keys.T[dt*128+p, :] (1024) | W_k[dt*128+p, :] (256) ]
CK_W = NK + 256
# packed "cq" layout: [ wq tiled (2*256) | qT tiled (2*64) ]
CQ_W = 2 * 256 + 2 * NQC
# packed "aux" input layout: [ vw (2) | va (8*258) ]
AUX_W = 2 + 8 * VA_W

_CACHED_NC = None


def build_kernel():
    """Build + compile the per-core Bass graph (SPMD across 8 cores)."""
    nc = bacc.Bacc("TRN2", target_bir_lowering=False, debug=False, num_devices=N_CORES)

    ck0_d = nc.declare_dram_parameter("ck0", [128, CK_W], BF16, isOutput=False)
    ck1_d = nc.declare_dram_parameter("ck1", [128, CK_W], BF16, isOutput=False)
    cq_d = nc.declare_dram_parameter("cq", [128, CQ_W], BF16, isOutput=False)
    aux_d = nc.declare_dram_parameter("aux", [128, AUX_W], BF16, isOutput=False)
    out_d = nc.declare_dram_parameter("out", [NQC, DV], F32, isOutput=True)

    Tanh = mybir.ActivationFunctionType.Tanh
    Exp = mybir.ActivationFunctionType.Exp

    with tile.TileContext(nc) as tc:
        with tc.tile_pool(name="const", bufs=1) as cpool:
            ck_sb = [cpool.tile([128, CK_W], BF16, tag=f"ck{dt}", name=f"ck{dt}") for dt in range(2)]
            cq_sb = cpool.tile([128, CQ_W], BF16)
            aux_sb = cpool.tile([128, AUX_W], BF16)
            nc.sync.dma_start(out=ck_sb[0], in_=ck0_d[:, :])
            nc.scalar.dma_start(out=ck_sb[1], in_=ck1_d[:, :])
            nc.sync.dma_start(out=cq_sb, in_=cq_d[:, :])
            nc.scalar.dma_start(out=aux_sb, in_=aux_d[:, :])

            def kT(dt, js):  # [128, len(js)] slice of keys^T, d-tile dt
                return ck_sb[dt][:, js.start : js.stop]

            def wk(dt, hs):
                return ck_sb[dt][:, NK + hs.start : NK + hs.stop]

            def wq(dt, hs):
                return cq_sb[:, dt * 256 + hs.start : dt * 256 + hs.stop]

            def qT(dt):
                return cq_sb[:, 2 * 256 + dt * NQC : 2 * 256 + (dt + 1) * NQC]

            vw_sb = aux_sb[:, 0:2]

            def va(jt):
                return aux_sb[:, 2 + jt * VA_W : 2 + (jt + 1) * VA_W]

            kh_sb = cpool.tile([128, 2, NK], BF16)
            qh_sb = cpool.tile([128, 2 * NQC], F32)
            zero_bias = cpool.tile([128, 1], F32)
            nc.vector.memset(zero_bias, 0.0)
            wT_sb = cpool.tile([128, 8, NQC], BF16)
            out_sb = cpool.tile([NQC, DV], F32)
            rsum = cpool.tile([NQC, 1], F32)

            # ---- P1: projections: kproj ht0 -> qproj -> kproj ht1, casts ----
            # ---- interleaved on DVE so the first adds can start early  ----
            with tc.tile_pool(name="proj_psum", bufs=2, space="PSUM") as pp:
                ps_k = pp.tile([128, 2 * NK], F32, tag="ps_k", bufs=1)
                ps_q = pp.tile([128, 2 * NQC], F32, tag="ps_q")

                def kproj(ht):
                    for jh in range(2):
                        for dt in range(2):
                            nc.tensor.matmul(
                                ps_k[:, ht * NK + jh * 512 : ht * NK + (jh + 1) * 512],
                                wk(dt, slice(ht * 128, (ht + 1) * 128)),
                                kT(dt, slice(jh * 512, (jh + 1) * 512)),
                                start=(dt == 0),
                                stop=(dt == 1),
                            )

                kproj(0)
                nc.vector.tensor_copy(kh_sb[:, 0, :], ps_k[:, 0:NK])
                for ht in range(2):
                    for dt in range(2):
                        nc.tensor.matmul(
                            ps_q[:, ht * NQC : (ht + 1) * NQC],
                            wq(dt, slice(ht * 128, (ht + 1) * 128)),
                            qT(dt),
                            start=(dt == 0),
                            stop=(dt == 1),
                        )
                nc.vector.tensor_copy(qh_sb, ps_q)
                kproj(1)
                nc.vector.tensor_copy(kh_sb[:, 1, :], ps_k[:, NK : 2 * NK])

            # ---- P2: q-add on VectorE (bf16 4x), giant tanh on ScalarE, ----
            # ----     transposed score accumulation on TensorE          ----
            with (
                tc.tile_pool(name="feat", bufs=2) as fpool,
                tc.tile_pool(name="sc_psum", bufs=1, space="PSUM") as spool,
            ):
                sT_t = [spool.tile([128, NQC], F32, tag=f"sT{jt}", name=f"sT{jt}") for jt in range(8)]
                # ramp block sizes: small blocks at the start (first tanh can
                # launch after few adds) and at the end (score-matmul tail).
                blocks = [1, 1, 2, 4] + [IB] * ((NQC - 16) // IB) + [4, 2, 1, 1]
                assert sum(blocks) == NQC
                i0 = 0
                for blk in blocks:
                    sums = fpool.tile([128, blk * 2 * NK], BF16, tag="sums", name="sums")
                    for ib in range(blk):
                        i = i0 + ib
                        for ht in range(2):
                            nc.vector.tensor_scalar_add(
                                sums[:, (ib * 2 + ht) * NK : (ib * 2 + ht + 1) * NK],
                                kh_sb[:, ht, :],
                                qh_sb[:, ht * NQC + i : ht * NQC + i + 1],
                            )
                    # tanh in place: sums tile becomes the feature tile
                    feat = sums
                    nc.scalar.activation(feat, sums, Tanh, bias=zero_bias, scale=1.0)
                    for ib in range(blk):
                        i = i0 + ib
                        for ht in range(2):
                            off = (ib * 2 + ht) * NK
                            for jt in range(8):
                                nc.tensor.matmul(
                                    sT_t[jt][:, i : i + 1],
                                    feat[:, off + jt * 128 : off + (jt + 1) * 128],
                                    vw_sb[:, ht : ht + 1],
                                    start=(ht == 0),
                                    stop=(ht == 1),
                                )
                    i0 += blk

                # ---- P3: exp straight out of PSUM into the transposed layout --
                for jt in range(8):
                    nc.scalar.activation(wT_sb[:, jt, :], sT_t[jt], Exp, bias=zero_bias, scale=1.0)

            # ---- P4/P5: weighted sum of (masked) values + normalize ----
            with tc.tile_pool(name="out_psum", bufs=1, space="PSUM") as opool:
                po = opool.tile([NQC, VA_W], F32)
                for jt in range(8):
                    nc.tensor.matmul(
                        po, wT_sb[:, jt, :], va(jt), start=(jt == 0), stop=(jt == 7)
                    )
                nc.vector.reciprocal(rsum, po[:, 256:257])
                nc.vector.tensor_scalar_mul(out_sb, po[:, 0:DV], rsum)
                nc.sync.dma_start(out=out_d[:, :], in_=out_sb)

    nc.compile()
    return nc


def _get_nc():
    global _CACHED_NC
    if _CACHED_NC is None:
        _CACHED_NC = build_kernel()
    return _CACHED_NC


def _tile128(x, n_tiles, width):
    """[n_tiles*128, width] -> [128, n_tiles*width] with [p, t*width+c] = x[t*128+p, c]."""
    return (
        np.transpose(np.ascontiguousarray(x, np.float32).reshape(n_tiles, 128, width), (1, 0, 2))
        .reshape(128, n_tiles * width)
    )


def make_in_maps(queries, keys, values, valid_lens, W_q, W_k, v_w):
    wk_f = np.asarray(W_k, np.float32)
    wq_p = _tile128(W_q, 2, H)
    vw_p = np.ascontiguousarray(np.asarray(v_w, np.float32).reshape(2, 128).T)
    in_maps = []
    for c in range(N_CORES):
        b, qhalf = divmod(c, 2)
        qs = np.asarray(queries[b, qhalf * NQC : (qhalf + 1) * NQC, :], np.float32)
        qT_p = _tile128(np.ascontiguousarray(qs.T), 2, NQC)
        kT = np.ascontiguousarray(np.asarray(keys[b], np.float32).T)  # [256, 1024]
        ck0 = np.concatenate([kT[:128], wk_f[:128]], axis=1).astype(NP_BF16)
        ck1 = np.concatenate([kT[128:], wk_f[128:]], axis=1).astype(NP_BF16)
        cq = np.concatenate([wq_p, qT_p], axis=1).astype(NP_BF16)

        vl = int(valid_lens[b])
        va = np.zeros((NK, VA_W), np.float32)
        va[:vl, :DV] = values[b, :vl]
        va[:vl, DV] = 1.0
        aux = np.concatenate([vw_p, _tile128(va, 8, VA_W)], axis=1).astype(NP_BF16)
        in_maps.append({"ck0": ck0, "ck1": ck1, "cq": cq, "aux": aux})
    return in_maps


def run(inputs, trace=False, **kwargs):
    nc = _get_nc()
    in_maps = make_in_maps(**inputs)
    res = run_bass_kernel_spmd(
        nc, in_maps, core_ids=list(range(N_CORES)), trace=trace, **kwargs
    )
    out = np.empty((B, NQ, DV), np.float32)
    for c in range(N_CORES):
        b, qhalf = divmod(c, 2)
        out[b, qhalf * NQC : (qhalf + 1) * NQC, :] = res.results[c]["out"]
    return out, res


def kernel(queries, keys, values, valid_lens, W_q, W_k, v_w):
    out, _ = run(
        dict(
            queries=queries,
            keys=keys,
            values=values,
            valid_lens=valid_lens,
            W_q=W_q,
            W_k=W_k,
            v_w=v_w,
        )
    )
    return out


# revision 27
# speedup vs baseline: 1.1863x; 1.1863x over previous
"""AdditiveAttention kernel for one TRN2 chip (8 NeuronCores).

Reference computation (per batch b):
    q = queries @ W_q                         # (NQ, H)
    k = keys @ W_k                            # (NK, H)
    scores[i,j] = sum_h v_w[h] * tanh(q[i,h] + k[j,h])
    out = masked_softmax(scores, valid_len) @ values

Sharding: data-parallel over (batch, query-half): core c handles batch c//2,
query rows (c%2)*64 .. +64.  All compute is core-local (no collectives);
the host does layout prep (transposes / masking / padding) and reassembly.

Device dataflow per core (64 queries x 1024 keys x H=256):
  P1  kh[h,j] = W_k.T @ keys.T, qh[h,i] = W_q.T @ queries.T     (PE, bf16)
  P2  sums = kh + qh[:,i]   (VectorE tensor_scalar, bf16 4x mode)
      feat = tanh(sums)     (ScalarE, giant FD to amortize overhead)
      scoresT[j,i] += feat[h,jblk].T @ v_w  (PE, feat as stationary operand)
      -> scores accumulate TRANSPOSED in PSUM, [j,i] layout
  P3  wT = exp(scoresT)  (no max subtraction needed: |score| <= sum|v_w| ~ 13)
  P4  out_aug[i,:] = sum_j wT[j,i] * va[j,:]  where va = [masked values | mask]
      -> column 256 of out_aug is the softmax denominator
  P5  out = out_aug[:, :256] * (1 / out_aug[:, 256])
"""

import numpy as np
import ml_dtypes

import concourse.bass as bass
import concourse.tile as tile
from concourse import bacc, mybir
from concourse.bass_utils import run_bass_kernel_spmd

BF16 = mybir.dt.bfloat16
F32 = mybir.dt.float32
NP_BF16 = ml_dtypes.bfloat16

B, NQ, NK, DQ, DK, H, DV = 4, 128, 1024, 256, 256, 256, 256
NQC = NQ // 2  # queries per core
VA_W = 258  # values (256) + mask column (1) + pad (1)
N_CORES = 8
IB = 8  # queries per tanh block

# packed "ck{dt}" layout (per partition row p = d % 128, bf16):
#   [ keys.T[dt*128+p, :] (1024) | W_k[dt*128+p, :] (256) ]
CK_W = NK + 256
# packed "cq" layout: [ wq tiled (2*256) | qT tiled (2*64) ]
CQ_W = 2 * 256 + 2 * NQC
# packed "aux" input layout: [ vw (2) | va (8*258) ]
AUX_W = 2 + 8 * VA_W

_CACHED_NC = None


def build_kernel():
    """Build + compile the per-core Bass graph (SPMD across 8 cores)."""
    nc = bacc.Bacc("TRN2", target_bir_lowering=False, debug=False, num_devices=N_CORES)

    ck0_d = nc.declare_dram_parameter("ck0", [128, CK_W], BF16, isOutput=False)
    ck1_d = nc.declare_dram_parameter("ck1", [128, CK_W], BF16, isOutput=False)
    cq_d = nc.declare_dram_parameter("cq", [128, CQ_W], BF16, isOutput=False)
    aux_d = nc.declare_dram_parameter("aux", [128, AUX_W], BF16, isOutput=False)
    out_d = nc.declare_dram_parameter("out", [NQC, DV], F32, isOutput=True)

    Tanh = mybir.ActivationFunctionType.Tanh
    Exp = mybir.ActivationFunctionType.Exp

    with tile.TileContext(nc) as tc:
        with tc.tile_pool(name="const", bufs=1) as cpool:
            ck_sb = [cpool.tile([128, CK_W], BF16, tag=f"ck{dt}", name=f"ck{dt}") for dt in range(2)]
            cq_sb = cpool.tile([128, CQ_W], BF16)
            aux_sb = cpool.tile([128, AUX_W], BF16)
            nc.sync.dma_start(out=ck_sb[0], in_=ck0_d[:, :])
            nc.scalar.dma_start(out=ck_sb[1], in_=ck1_d[:, :])
            nc.sync.dma_start(out=cq_sb, in_=cq_d[:, :])
            nc.scalar.dma_start(out=aux_sb, in_=aux_d[:, :])

            def kT(dt, js):  # [128, len(js)] slice of keys^T, d-tile dt
                return ck_sb[dt][:, js.start : js.stop]

            def wk(dt, hs):
                return ck_sb[dt][:, NK + hs.start : NK + hs.stop]

            def wq(dt, hs):
                return cq_sb[:, dt * 256 + hs.start : dt * 256 + hs.stop]

            def qT(dt):
                return cq_sb[:, 2 * 256 + dt * NQC : 2 * 256 + (dt + 1) * NQC]

            vw_sb = aux_sb[:, 0:2]

            def va(jt):
                return aux_sb[:, 2 + jt * VA_W : 2 + (jt + 1) * VA_W]

            kh_sb = cpool.tile([128, 2, NK], BF16)
            qh_sb = cpool.tile([128, 2 * NQC], F32)
            zero_bias = cpool.tile([128, 1], F32)
            nc.vector.memset(zero_bias, 0.0)
            wT_sb = cpool.tile([128, 8, NQC], BF16)
            out_sb = cpool.tile([NQC, DV], F32)
            rsum = cpool.tile([NQC, 1], F32)

            # ---- P1: projections: kproj ht0 -> qproj -> kproj ht1, casts ----
            # ---- interleaved on DVE so the first adds can start early  ----
            with tc.tile_pool(name="proj_psum", bufs=2, space="PSUM") as pp:
                ps_k = pp.tile([128, 2 * NK], F32, tag="ps_k", bufs=1)
                ps_q = pp.tile([128, 2 * NQC], F32, tag="ps_q")

                def kproj(ht):
                    for jh in range(2):
                        for dt in range(2):
                            nc.tensor.matmul(
                                ps_k[:, ht * NK + jh * 512 : ht * NK + (jh + 1) * 512],
                                wk(dt, slice(ht * 128, (ht + 1) * 128)),
                                kT(dt, slice(jh * 512, (jh + 1) * 512)),
                                start=(dt == 0),
                                stop=(dt == 1),
                            )

                kproj(0)
                nc.vector.tensor_copy(kh_sb[:, 0, :], ps_k[:, 0:NK])
                for ht in range(2):
                    for dt in range(2):
                        nc.tensor.matmul(
                            ps_q[:, ht * NQC : (ht + 1) * NQC],
                            wq(dt, slice(ht * 128, (ht + 1) * 128)),
                            qT(dt),
                            start=(dt == 0),
                            stop=(dt == 1),
                        )
                nc.vector.tensor_copy(qh_sb, ps_q)
                kproj(1)
                nc.vector.tensor_copy(kh_sb[:, 1, :], ps_k[:, NK : 2 * NK])

            # ---- P2: q-add on VectorE (bf16 4x), giant tanh on ScalarE, ----
            # ----     transposed score accumulation on TensorE          ----
            with (
                tc.tile_pool(name="feat", bufs=2) as fpool,
                tc.tile_pool(name="sc_psum", bufs=1, space="PSUM") as spool,
            ):
                sT_t = [spool.tile([128, NQC], F32, tag=f"sT{jt}", name=f"sT{jt}") for jt in range(8)]
                # ramp block sizes: small blocks at the start (first tanh can
                # launch after few adds) and at the end (score-matmul tail).
                blocks = [1, 1, 2, 4] + [IB] * ((NQC - 16) // IB) + [4, 2, 1, 1]
                assert sum(blocks) == NQC
                i0 = 0
                for blk in blocks:
                    sums = fpool.tile([128, blk * 2 * NK], BF16, tag="sums", name="sums", bufs=3)
                    for ib in range(blk):
                        i = i0 + ib
                        for ht in range(2):
                            nc.vector.tensor_scalar_add(
                                sums[:, (ib * 2 + ht) * NK : (ib * 2 + ht + 1) * NK],
                                kh_sb[:, ht, :],
                                qh_sb[:, ht * NQC + i : ht * NQC + i + 1],
                            )
                    # tanh in place: sums tile becomes the feature tile
                    feat = sums
                    nc.scalar.activation(feat, sums, Tanh, bias=zero_bias, scale=1.0)
                    for ib in range(blk):
                        i = i0 + ib
                        for ht in range(2):
                            off = (ib * 2 + ht) * NK
                            for jt in range(8):
                                nc.tensor.matmul(
                                    sT_t[jt][:, i : i + 1],
                                    feat[:, off + jt * 128 : off + (jt + 1) * 128],
                                    vw_sb[:, ht : ht + 1],
                                    start=(ht == 0),
                                    stop=(ht == 1),
                                )
                    i0 += blk

                # ---- P3: exp straight out of PSUM into the transposed layout --
                for jt in range(8):
                    nc.scalar.activation(wT_sb[:, jt, :], sT_t[jt], Exp, bias=zero_bias, scale=1.0)

            # ---- P4/P5: weighted sum of (masked) values + normalize ----
            with tc.tile_pool(name="out_psum", bufs=1, space="PSUM") as opool:
                po = opool.tile([NQC, VA_W], F32)
                for jt in range(8):
                    nc.tensor.matmul(
                        po, wT_sb[:, jt, :], va(jt), start=(jt == 0), stop=(jt == 7)
                    )
                nc.vector.reciprocal(rsum, po[:, 256:257])
                nc.vector.tensor_scalar_mul(out_sb, po[:, 0:DV], rsum)
                nc.sync.dma_start(out=out_d[:, :], in_=out_sb)

    nc.compile()
    return nc


def _get_nc():
    global _CACHED_NC
    if _CACHED_NC is None:
        _CACHED_NC = build_kernel()
    return _CACHED_NC


def _tile128(x, n_tiles, width):
    """[n_tiles*128, width] -> [128, n_tiles*width] with [p, t*width+c] = x[t*128+p, c]."""
    return (
        np.transpose(np.ascontiguousarray(x, np.float32).reshape(n_tiles, 128, width), (1, 0, 2))
        .reshape(128, n_tiles * width)
    )


def make_in_maps(queries, keys, values, valid_lens, W_q, W_k, v_w):
    wk_f = np.asarray(W_k, np.float32)
    wq_p = _tile128(W_q, 2, H)
    vw_p = np.ascontiguousarray(np.asarray(v_w, np.float32).reshape(2, 128).T)
    in_maps = []
    for c in range(N_CORES):
        b, qhalf = divmod(c, 2)
        qs = np.asarray(queries[b, qhalf * NQC : (qhalf + 1) * NQC, :], np.float32)
        qT_p = _tile128(np.ascontiguousarray(qs.T), 2, NQC)
        kT = np.ascontiguousarray(np.asarray(keys[b], np.float32).T)  # [256, 1024]
        ck0 = np.concatenate([kT[:128], wk_f[:128]], axis=1).astype(NP_BF16)
        ck1 = np.concatenate([kT[128:], wk_f[128:]], axis=1).astype(NP_BF16)
        cq = np.concatenate([wq_p, qT_p], axis=1).astype(NP_BF16)

        vl = int(valid_lens[b])
        va = np.zeros((NK, VA_W), np.float32)
        va[:vl, :DV] = values[b, :vl]
        va[:vl, DV] = 1.0
        aux = np.concatenate([vw_p, _tile128(va, 8, VA_W)], axis=1).astype(NP_BF16)
        in_maps.append({"ck0": ck0, "ck1": ck1, "cq": cq, "aux": aux})
    return in_maps


def run(inputs, trace=False, **kwargs):
    nc = _get_nc()
    in_maps = make_in_maps(**inputs)
    res = run_bass_kernel_spmd(
        nc, in_maps, core_ids=list(range(N_CORES)), trace=trace, **kwargs
    )
    out = np.empty((B, NQ, DV), np.float32)
    for c in range(N_CORES):
        b, qhalf = divmod(c, 2)
        out[b, qhalf * NQC : (qhalf + 1) * NQC, :] = res.results[c]["out"]
    return out, res


def kernel(queries, keys, values, valid_lens, W_q, W_k, v_w):
    out, _ = run(
        dict(
            queries=queries,
            keys=keys,
            values=values,
            valid_lens=valid_lens,
            W_q=W_q,
            W_k=W_k,
            v_w=v_w,
        )
    )
    return out


# revision 35
# speedup vs baseline: 1.2056x; 1.0163x over previous
"""AdditiveAttention kernel for one TRN2 chip (8 NeuronCores).

Reference computation (per batch b):
    q = queries @ W_q                         # (NQ, H)
    k = keys @ W_k                            # (NK, H)
    scores[i,j] = sum_h v_w[h] * tanh(q[i,h] + k[j,h])
    out = masked_softmax(scores, valid_len) @ values

Sharding: data-parallel over (batch, query-half): core c handles batch c//2,
query rows (c%2)*64 .. +64.  All compute is core-local (no collectives);
the host does layout prep (transposes / masking / padding) and reassembly.

Device dataflow per core (64 queries x 1024 keys x H=256):
  P1  kh[h,j] = W_k.T @ keys.T, qh[h,i] = W_q.T @ queries.T     (PE, bf16)
  P2  sums = kh + qh[:,i]   (VectorE tensor_scalar, bf16 4x mode)
      feat = tanh(sums)     (ScalarE, giant FD to amortize overhead)
      scoresT[j,i] += feat[h,jblk].T @ v_w  (PE, feat as stationary operand)
      -> scores accumulate TRANSPOSED in PSUM, [j,i] layout
  P3  wT = exp(scoresT)  (no max subtraction needed: |score| <= sum|v_w| ~ 13)
  P4  out_aug[i,:] = sum_j wT[j,i] * va[j,:]  where va = [masked values | mask]
      -> column 256 of out_aug is the softmax denominator
  P5  out = out_aug[:, :256] * (1 / out_aug[:, 256])
"""

import numpy as np
import ml_dtypes

import concourse.bass as bass
import concourse.tile as tile
from concourse import bacc, mybir
from concourse.bass_utils import run_bass_kernel_spmd

BF16 = mybir.dt.bfloat16
F32 = mybir.dt.float32
NP_BF16 = ml_dtypes.bfloat16

B, NQ, NK, DQ, DK, H, DV = 4, 128, 1024, 256, 256, 256, 256
NQC = NQ // 2  # queries per core
VA_W = 258  # values (256) + mask column (1) + pad (1)
N_CORES = 8
IB = 8  # queries per tanh block

# packed "cka{dt}" layout (per partition row p, d = dt*128+p, bf16):
#   [ keys.T[d, 0:512] | W_k[d, :] (256) ]   -> first key-half + weights
# packed "ckb{dt}": [ keys.T[d, 512:1024] ]  -> second key-half
CKA_W = 512 + 256
CKB_W = 512
# packed "cq" layout: [ wq tiled (2*256) | qT tiled (2*64) ]
CQ_W = 2 * 256 + 2 * NQC
# packed "aux" input layout: [ vw (2) | va (8*258) ]
AUX_W = 2 + 8 * VA_W

_CACHED_NC = None


def build_kernel():
    """Build + compile the per-core Bass graph (SPMD across 8 cores)."""
    nc = bacc.Bacc("TRN2", target_bir_lowering=False, debug=False, num_devices=N_CORES)

    cka_d = [
        nc.declare_dram_parameter(f"cka{dt}", [128, CKA_W], BF16, isOutput=False)
        for dt in range(2)
    ]
    ckb_d = [
        nc.declare_dram_parameter(f"ckb{dt}", [128, CKB_W], BF16, isOutput=False)
        for dt in range(2)
    ]
    cq_d = nc.declare_dram_parameter("cq", [128, CQ_W], BF16, isOutput=False)
    aux_d = nc.declare_dram_parameter("aux", [128, AUX_W], BF16, isOutput=False)
    out_d = nc.declare_dram_parameter("out", [NQC, DV], F32, isOutput=True)

    Tanh = mybir.ActivationFunctionType.Tanh
    Exp = mybir.ActivationFunctionType.Exp

    with tile.TileContext(nc) as tc:
        with tc.tile_pool(name="const", bufs=1) as cpool:
            cka_sb = [cpool.tile([128, CKA_W], BF16, tag=f"cka{dt}", name=f"cka{dt}") for dt in range(2)]
            ckb_sb = [cpool.tile([128, CKB_W], BF16, tag=f"ckb{dt}", name=f"ckb{dt}") for dt in range(2)]
            cq_sb = cpool.tile([128, CQ_W], BF16)
            aux_sb = cpool.tile([128, AUX_W], BF16)
            # sync HWDGE issues right after preamble; scalar HWDGE only after
            # the ACT table load (~8.5us) -- keep both queues busy in parallel
            nc.sync.dma_start(out=cka_sb[0], in_=cka_d[0][:, :])
            nc.scalar.dma_start(out=cka_sb[1], in_=cka_d[1][:, :])
            nc.sync.dma_start(out=ckb_sb[0], in_=ckb_d[0][:, :])
            nc.scalar.dma_start(out=ckb_sb[1], in_=ckb_d[1][:, :])
            nc.sync.dma_start(out=cq_sb, in_=cq_d[:, :])
            nc.scalar.dma_start(out=aux_sb, in_=aux_d[:, :])

            def kT(dt, jh):  # [128, 512] slice of keys^T, d-tile dt, key-half jh
                return cka_sb[dt][:, 0:512] if jh == 0 else ckb_sb[dt][:, 0:512]

            def wk(dt, hs):
                return cka_sb[dt][:, 512 + hs.start : 512 + hs.stop]

            def wq(dt, hs):
                return cq_sb[:, dt * 256 + hs.start : dt * 256 + hs.stop]

            def qT(dt):
                return cq_sb[:, 2 * 256 + dt * NQC : 2 * 256 + (dt + 1) * NQC]

            vw_sb = aux_sb[:, 0:2]

            def va(jt):
                return aux_sb[:, 2 + jt * VA_W : 2 + (jt + 1) * VA_W]

            kh_sb = cpool.tile([128, 2, NK], BF16)
            qh_sb = cpool.tile([128, 2 * NQC], F32)
            zero_bias = cpool.tile([128, 1], F32)
            nc.vector.memset(zero_bias, 0.0)
            wT_sb = cpool.tile([128, 8, NQC], BF16)
            out_sb = cpool.tile([NQC, DV], F32)
            rsum = cpool.tile([NQC, 1], F32)

            # ---- P1: projections: kproj ht0 -> qproj -> kproj ht1, casts ----
            # ---- interleaved on DVE so the first adds can start early  ----
            with tc.tile_pool(name="proj_psum", bufs=2, space="PSUM") as pp:
                ps_k = pp.tile([128, 2 * NK], F32, tag="ps_k", bufs=1)
                ps_q = pp.tile([128, 2 * NQC], F32, tag="ps_q")

                def kproj(ht):
                    for jh in range(2):
                        for dt in range(2):
                            nc.tensor.matmul(
                                ps_k[:, ht * NK + jh * 512 : ht * NK + (jh + 1) * 512],
                                wk(dt, slice(ht * 128, (ht + 1) * 128)),
                                kT(dt, jh),
                                start=(dt == 0),
                                stop=(dt == 1),
                            )

                kproj(0)
                nc.vector.tensor_copy(kh_sb[:, 0, :], ps_k[:, 0:NK])
                for ht in range(2):
                    for dt in range(2):
                        nc.tensor.matmul(
                            ps_q[:, ht * NQC : (ht + 1) * NQC],
                            wq(dt, slice(ht * 128, (ht + 1) * 128)),
                            qT(dt),
                            start=(dt == 0),
                            stop=(dt == 1),
                        )
                nc.vector.tensor_copy(qh_sb, ps_q)
                kproj(1)
                nc.vector.tensor_copy(kh_sb[:, 1, :], ps_k[:, NK : 2 * NK])

            # ---- P2: q-add on VectorE (bf16 4x), giant tanh on ScalarE, ----
            # ----     transposed score accumulation on TensorE          ----
            with (
                tc.tile_pool(name="feat", bufs=2) as fpool,
                tc.tile_pool(name="sc_psum", bufs=1, space="PSUM") as spool,
            ):
                sT_t = [spool.tile([128, NQC], F32, tag=f"sT{jt}", name=f"sT{jt}") for jt in range(8)]
                # ramp block sizes: small blocks at the start (first tanh can
                # launch after few adds) and at the end (score-matmul tail).
                blocks = [1, 1, 2, 4] + [IB] * ((NQC - 16) // IB) + [4, 2, 1, 1]
                assert sum(blocks) == NQC
                i0 = 0
                for blk in blocks:
                    sums = fpool.tile([128, blk * 2 * NK], BF16, tag="sums", name="sums")
                    for ib in range(blk):
                        i = i0 + ib
                        for ht in range(2):
                            nc.vector.tensor_scalar_add(
                                sums[:, (ib * 2 + ht) * NK : (ib * 2 + ht + 1) * NK],
                                kh_sb[:, ht, :],
                                qh_sb[:, ht * NQC + i : ht * NQC + i + 1],
                            )
                    feat = fpool.tile([128, blk * 2 * NK], BF16, tag="feat", name="feat")
                    nc.scalar.activation(feat, sums, Tanh, bias=zero_bias, scale=1.0)
                    for ib in range(blk):
                        i = i0 + ib
                        for ht in range(2):
                            off = (ib * 2 + ht) * NK
                            for jt in range(8):
                                nc.tensor.matmul(
                                    sT_t[jt][:, i : i + 1],
                                    feat[:, off + jt * 128 : off + (jt + 1) * 128],
                                    vw_sb[:, ht : ht + 1],
                                    start=(ht == 0),
                                    stop=(ht == 1),
                                )
                    i0 += blk

                # ---- P3: exp straight out of PSUM into the transposed layout --
                for jt in range(8):
                    nc.scalar.activation(wT_sb[:, jt, :], sT_t[jt], Exp, bias=zero_bias, scale=1.0)

            # ---- P4/P5: weighted sum of (masked) values + normalize ----
            with tc.tile_pool(name="out_psum", bufs=1, space="PSUM") as opool:
                po = opool.tile([NQC, VA_W], F32)
                for jt in range(8):
                    nc.tensor.matmul(
                        po, wT_sb[:, jt, :], va(jt), start=(jt == 0), stop=(jt == 7)
                    )
                nc.vector.reciprocal(rsum, po[:, 256:257])
                nc.vector.tensor_scalar_mul(out_sb, po[:, 0:DV], rsum)
                nc.sync.dma_start(out=out_d[:, :], in_=out_sb)

    nc.compile()
    return nc


def _get_nc():
    global _CACHED_NC
    if _CACHED_NC is None:
        _CACHED_NC = build_kernel()
    return _CACHED_NC


def _tile128(x, n_tiles, width):
    """[n_tiles*128, width] -> [128, n_tiles*width] with [p, t*width+c] = x[t*128+p, c]."""
    return (
        np.transpose(np.ascontiguousarray(x, np.float32).reshape(n_tiles, 128, width), (1, 0, 2))
        .reshape(128, n_tiles * width)
    )


def make_in_maps(queries, keys, values, valid_lens, W_q, W_k, v_w):
    wk_f = np.asarray(W_k, np.float32)
    wq_p = _tile128(W_q, 2, H)
    vw_p = np.ascontiguousarray(np.asarray(v_w, np.float32).reshape(2, 128).T)
    in_maps = []
    for c in range(N_CORES):
        b, qhalf = divmod(c, 2)
        qs = np.asarray(queries[b, qhalf * NQC : (qhalf + 1) * NQC, :], np.float32)
        qT_p = _tile128(np.ascontiguousarray(qs.T), 2, NQC)
        kT = np.ascontiguousarray(np.asarray(keys[b], np.float32).T)  # [256, 1024]
        cka0 = np.concatenate([kT[:128, :512], wk_f[:128]], axis=1).astype(NP_BF16)
        cka1 = np.concatenate([kT[128:, :512], wk_f[128:]], axis=1).astype(NP_BF16)
        ckb0 = np.ascontiguousarray(kT[:128, 512:]).astype(NP_BF16)
        ckb1 = np.ascontiguousarray(kT[128:, 512:]).astype(NP_BF16)
        cq = np.concatenate([wq_p, qT_p], axis=1).astype(NP_BF16)

        vl = int(valid_lens[b])
        va = np.zeros((NK, VA_W), np.float32)
        va[:vl, :DV] = values[b, :vl]
        va[:vl, DV] = 1.0
        aux = np.concatenate([vw_p, _tile128(va, 8, VA_W)], axis=1).astype(NP_BF16)
        in_maps.append(
            {"cka0": cka0, "cka1": cka1, "ckb0": ckb0, "ckb1": ckb1, "cq": cq, "aux": aux}
        )
    return in_maps


def run(inputs, trace=False, **kwargs):
    nc = _get_nc()
    in_maps = make_in_maps(**inputs)
    res = run_bass_kernel_spmd(
        nc, in_maps, core_ids=list(range(N_CORES)), trace=trace, **kwargs
    )
    out = np.empty((B, NQ, DV), np.float32)
    for c in range(N_CORES):
        b, qhalf = divmod(c, 2)
        out[b, qhalf * NQC : (qhalf + 1) * NQC, :] = res.results[c]["out"]
    return out, res


def kernel(queries, keys, values, valid_lens, W_q, W_k, v_w):
    out, _ = run(
        dict(
            queries=queries,
            keys=keys,
            values=values,
            valid_lens=valid_lens,
            W_q=W_q,
            W_k=W_k,
            v_w=v_w,
        )
    )
    return out
